# revision 2
# baseline (speedup 1.0000x reference)
"""Trainium2 Bass kernel for a GNN message-passing layer (GCL).

reference:
    m   = relu(concat(h[row], h[col]) @ edge_w + edge_b)       # [E, H]
    agg = segment_sum(m, row, N)                               # [N, H]
    out = relu(concat(h, agg) @ node_w + node_b)               # [N, H]

Strategy (8 cores, edge-parallel with node-range ownership, no collectives):
  * Precompute AB[n] = [h[n] @ Wtop + b | h[n] @ Wbot]  (bf16, DRAM table).
    Then m_e = relu(A[row_e] + B[col_e]).
  * B[col_e] comes from one dma_gather per edge (256B elems, the descriptor
    count is the bottleneck).  The old row-side gather is eliminated: A for
    the core's own 12544 nodes lives in SBUF ([128, W, 64] bf16) and the
    per-edge A is expanded on-chip with a one-hot matmul
        A_exp = oh2^T @ A_window,   oh2[n, e] = (row_local[e] == n)
    where oh2 is built by an is_equal into a 32x32-block-permuted layout and
    a DVE stream-transpose (the only cross-partition shuffle DVE can do).
  * Segment-sum via one-hot matmul as before: PE accumulates m.T @ oh.
  * Node MLP with bias folded in via an appended ones-row (K=65 matmul).
Each core owns a contiguous 12544-node range; rows of its edges fall in that
range, so aggregation and the node MLP are fully local.
"""

import math
import numpy as np
import ml_dtypes

import concourse.bass as bass
import concourse.bacc as bacc
import concourse.tile as tile
from concourse import mybir
from concourse.tile import TileContext
from concourse.library_config import mlp as mlp_library

BF16 = mybir.dt.bfloat16
F32 = mybir.dt.float32
I16 = mybir.dt.int16
NP_BF16 = ml_dtypes.bfloat16


class Cfg:
    def __init__(self, n_nodes, n_cores=8, spc=7, table_f32=False):
        self.n_swdge_queues = 4   # parallel SWDGE queues
        self.N = n_nodes
        self.n_cores = n_cores
        self.NPC = int(math.ceil(n_nodes / n_cores / 128)) * 128
        self.NP = self.NPC * n_cores
        self.W = self.NPC // 128          # windows per core
        self.C = 4                        # col chunks (int16 gather idx limit)
        assert self.NP % self.C == 0
        self.CHUNK = self.NP // self.C
        assert self.CHUNK <= 32767, "int16 gather index limit"
        # segments (windows) per gather call; must divide W
        self.SPC = spc
        assert self.W % self.SPC == 0
        self.CALLS_PER_CHUNK = self.W // self.SPC
        # idx loads cover IDX_CALLS gather calls each
        self.IDX_CALLS = self.CALLS_PER_CHUNK // 2 if self.CALLS_PER_CHUNK % 2 == 0 else self.CALLS_PER_CHUNK
        self.SEG = None  # set from data

    def stripe(self, total):
        for cand in (8192, 6272, 4096, 3136, 2048, 1792, 1568, 1024, 896, 784, 512, 448, 256, 128):
            if cand <= total and total % cand == 0:
                return cand
        raise AssertionError(total)


def build_kernel(cfg, phases=(0, 1, 2), p1_level=4, p2_level=3):
    """Build the single-core SPMD program. Returns nc.
    p1_level: 1=gathers only, 2=+onehots (DVE), 3=+A-mm/add/relu, 4=full."""
    SEG = cfg.SEG
    assert SEG is not None and SEG % 128 == 0
    EP = cfg.C * cfg.W * SEG               # padded edges per core
    NCALL = cfg.SPC * SEG                  # idxs per gather call
    JPC = NCALL // 128                     # 128-chunks per call
    JPS = SEG // 128                       # 128-chunks per segment
    # >64 descriptors/engine in one packet wedges the device; the per-engine
    # descriptor count is NCALL//16 + 1.
    SINGLE_PACKET = (NCALL // 16 + 1) <= 64

    NSWQ = getattr(cfg, "n_swdge_queues", 1)
    nc = bacc.Bacc("TRN2", target_bir_lowering=False, debug=False,
                   num_swdge_queues=NSWQ)

    # ---- DRAM I/O ----
    hTa_d = nc.dram_tensor("hTa", [65, cfg.NP], BF16, kind="ExternalInput")
    hTownb_d = nc.dram_tensor("hTownb", [65, cfg.NPC], BF16, kind="ExternalInput")
    hTown_d = nc.dram_tensor("hTown", [65, cfg.NPC], F32, kind="ExternalInput")
    waug_d = nc.dram_tensor("waug", [65, 128], BF16, kind="ExternalInput")
    nw1_d = nc.dram_tensor("nw1", [64, 64], F32, kind="ExternalInput")
    nw2a_d = nc.dram_tensor("nw2a", [65, 64], F32, kind="ExternalInput")
    iota_d = nc.dram_tensor("iota", [128, 128], BF16, kind="ExternalInput")
    iota_st_d = nc.dram_tensor("iota_st", [128, 32], BF16, kind="ExternalInput")
    colidx_d = nc.dram_tensor("colidx", [128, EP // 16], I16, kind="ExternalInput")
    rl_d = nc.dram_tensor("rl", [128, EP // 128], BF16, kind="ExternalInput")
    rl4_d = nc.dram_tensor("rl4", [128, EP // 32], BF16, kind="ExternalInput")
    AB_ds = [nc.dram_tensor(f"AB{c}", [cfg.CHUNK, 128], BF16)
             for c in range(cfg.C)]
    out_d = nc.dram_tensor("out", [cfg.NPC, 64], F32, kind="ExternalOutput")

    with TileContext(nc) as tc:
        nc.gpsimd.load_library(mlp_library)

        with tc.tile_pool(name="const", bufs=1) as cpool:
            waug_sb = cpool.tile([65, 128], BF16)
            nc.sync.dma_start(out=waug_sb[:], in_=waug_d[:])
            iota_sb = cpool.tile([128, 128], BF16)
            nc.sync.dma_start(out=iota_sb[:], in_=iota_d[:])
            iota_st_sb = cpool.tile([128, 32], BF16)
            nc.sync.dma_start(out=iota_st_sb[:], in_=iota_st_d[:])
            nw1_sb = cpool.tile([64, 64], F32)
            nc.sync.dma_start(out=nw1_sb[:], in_=nw1_d[:])
            nw2a_sb = cpool.tile([65, 64], F32)
            nc.sync.dma_start(out=nw2a_sb[:], in_=nw2a_d[:])

            # aggT arena [65, NPC]: rows 0:64 = aggT, row 64 = ones (bias row)
            arena = cpool.tile([65, cfg.NPC], F32)
            nc.vector.memset(arena[64:65, :], 1.0)

            # A table for own nodes, SBUF-resident: [128, W, 64] bf16
            aown_sb = cpool.tile([128, cfg.W, 64], BF16)

            # ---- Phase 0: build AB table (all NP nodes) + Aown (SBUF) ----
            if 0 in phases:
                with tc.tile_pool(name="p0a", bufs=1) as p0a, \
                     tc.tile_pool(name="p0aps", bufs=4, space="PSUM") as p0aps:
                    htb = p0a.tile([65, cfg.NPC], BF16)
                    nc.sync.dma_start(out=htb[:], in_=hTownb_d[:])
                    for w in range(cfg.W):
                        ps = p0aps.tile([128, 64], F32)
                        nc.tensor.matmul(
                            out=ps[:], lhsT=htb[:, w * 128:(w + 1) * 128],
                            rhs=waug_sb[:, 0:64], start=True, stop=True)
                        nc.vector.tensor_copy(out=aown_sb[:, w, :], in_=ps[:])

                SN = cfg.stripe(cfg.CHUNK)
                JT = SN // 128
                with tc.tile_pool(name="p0", bufs=2) as p0, \
                     tc.tile_pool(name="p0ps", bufs=4, space="PSUM") as p0ps:
                    for s in range(cfg.NP // SN):
                        hstripe = p0.tile([65, SN], BF16, tag="hstripe")
                        nc.sync.dma_start(
                            out=hstripe[:], in_=hTa_d[:, s * SN:(s + 1) * SN])
                        abst = p0.tile([128, JT, 128], BF16, tag="abst")
                        for j in range(JT):
                            ps = p0ps.tile([128, 128], F32)
                            nc.tensor.matmul(
                                out=ps[:], lhsT=hstripe[:, j * 128:(j + 1) * 128],
                                rhs=waug_sb[:], start=True, stop=True)
                            nc.vector.tensor_copy(out=abst[:, j, :], in_=ps[:])
                        n0 = s * SN
                        dst_d, off = AB_ds[n0 // cfg.CHUNK], n0 % cfg.CHUNK
                        nc.sync.dma_start(
                            out=dst_d[off:off + SN, :].rearrange(
                                "(j p) f -> p j f", p=128),
                            in_=abst[:])

            # ---- Phase 1: gather + edge MLP + one-hot aggregation ----
            if 1 in phases:
              with tc.tile_pool(name="rlp", bufs=1) as rlp:
                rl_sb = rlp.tile([128, EP // 128], BF16)
                nc.sync.dma_start(out=rl_sb[:], in_=rl_d[:])
                rl4_sb = rlp.tile([128, EP // 32], BF16)
                nc.sync.dma_start(out=rl4_sb[:], in_=rl4_d[:])

                with tc.tile_pool(name="idxp", bufs=2) as idxp, \
                     tc.tile_pool(name="gath", bufs=3) as gathp, \
                     tc.tile_pool(name="stp", bufs=3) as stp, \
                     tc.tile_pool(name="oh2p", bufs=3) as oh2p, \
                     tc.tile_pool(name="ohp", bufs=3) as ohp, \
                     tc.tile_pool(name="mp", bufs=3) as mp, \
                     tc.tile_pool(name="psA", bufs=4, space="PSUM") as psAp, \
                     tc.tile_pool(name="p1ps", bufs=4, space="PSUM") as p1ps:
                    IC = cfg.IDX_CALLS
                    ILEN = IC * NCALL // 16      # idx cols per load
                    for c in range(cfg.C):
                        for g in range(cfg.CALLS_PER_CHUNK // IC):
                            goff = (c * cfg.CALLS_PER_CHUNK + g * IC) * NCALL // 16
                            cidx = idxp.tile([128, ILEN], I16, tag="cidx")
                            nc.sync.dma_start(
                                out=cidx[:], in_=colidx_d[:, goff:goff + ILEN])
                            for cl in range(IC):
                                call = (c * cfg.CALLS_PER_CHUNK + g * IC + cl)
                                colg = gathp.tile([128, JPC, 128], BF16, tag="g")
                                nc.gpsimd.dma_gather(
                                    colg[:], AB_ds[c][:],
                                    cidx[:, cl * (NCALL // 16):(cl + 1) * (NCALL // 16)],
                                    NCALL, NCALL, 128, single_packet=SINGLE_PACKET,
                                    queue_num=call % NSWQ)
                                for s in range(cfg.SPC if p1_level >= 2 else 0):
                                    w = (call % cfg.CALLS_PER_CHUNK) * cfg.SPC + s
                                    gseg = call * cfg.SPC + s
                                    j0 = s * JPS
                                    # one-hot, edge-partition [128e, JPS, 128n]
                                    oh = ohp.tile([128, JPS, 128], BF16, tag="oh")
                                    nc.vector.tensor_tensor(
                                        out=oh[:],
                                        in0=rl_sb[:, gseg * JPS:gseg * JPS + JPS]
                                            .to_broadcast([128, JPS, 128]),
                                        in1=iota_sb[:].rearrange("p (a b) -> p a b", a=1)
                                            .to_broadcast([128, JPS, 128]),
                                        op=mybir.AluOpType.is_equal)
                                    # block-permuted one-hot for stream transpose
                                    sti = stp.tile([128, JPS * 4, 32], BF16, tag="sti")
                                    q0 = gseg * JPS * 4
                                    nc.vector.tensor_tensor(
                                        out=sti[:],
                                        in0=rl4_sb[:, q0:q0 + JPS * 4]
                                            .to_broadcast([128, JPS * 4, 32]),
                                        in1=iota_st_sb[:].rearrange("p (a b) -> p a b", a=1)
                                            .to_broadcast([128, JPS * 4, 32]),
                                        op=mybir.AluOpType.is_equal)
                                    # oh2[n, e] node-partition one-hot
                                    oh2 = oh2p.tile([128, JPS, 128], BF16, tag="oh2")
                                    nc.vector.transpose(
                                        out=oh2[:].rearrange("p a b -> p (a b)"),
                                        in_=sti[:].rearrange("p a b -> p (a b)"))
                                    if p1_level < 3:
                                        continue
                                    madd = mp.tile([128, JPS, 64], BF16, tag="madd")
                                    for jj in range(0, JPS, 3):
                                        gs = min(3, JPS - jj)
                                        pa = psAp.tile([128, gs, 64], F32)
                                        for j in range(gs):
                                            nc.tensor.matmul(
                                                out=pa[:, j, :],
                                                lhsT=oh2[:, jj + j, :],
                                                rhs=aown_sb[:, w, :],
                                                start=True, stop=True)
                                        nc.vector.tensor_tensor(
                                            out=madd[:, jj:jj + gs, :],
                                            in0=pa[:],
                                            in1=colg[:, j0 + jj:j0 + jj + gs, 64:128],
                                            op=mybir.AluOpType.add)
                                    m2 = mp.tile([128, JPS, 64], BF16, tag="m2")
                                    nc.scalar.activation(
                                        out=m2[:], in_=madd[:],
                                        func=mybir.ActivationFunctionType.Relu)
                                    if p1_level < 4:
                                        continue
                                    ps = p1ps.tile([64, 128], F32)
                                    for j in range(JPS):
                                        nc.tensor.matmul(
                                            out=ps[:], lhsT=m2[:, j, :], rhs=oh[:, j, :],
                                            start=(j == 0), stop=(j == JPS - 1))
                                    dst = arena[0:64, w * 128:(w + 1) * 128]
                                    if c == 0:
                                        nc.vector.tensor_copy(out=dst, in_=ps[:])
                                    else:
                                        nc.vector.tensor_tensor(
                                            out=dst, in0=dst, in1=ps[:],
                                            op=mybir.AluOpType.add)

            # ---- Phase 2: node MLP ----
            if 2 in phases:
              with tc.tile_pool(name="p2", bufs=2) as p2, \
                 tc.tile_pool(name="p2ps", bufs=4, space="PSUM") as p2ps:
                GW = cfg.SPC                  # windows per output group
                for g in range(cfg.W // GW):
                    hT2 = p2.tile([65, GW * 128], F32, tag="hT2")
                    nc.sync.dma_start(
                        out=hT2[:],
                        in_=hTown_d[:, g * GW * 128:(g + 1) * GW * 128])
                    ost = p2.tile([128, GW, 64], F32, tag="ost")
                    for i in range(GW):
                        w = g * GW + i
                        if p2_level < 2:
                            nc.vector.memset(ost[:, i, :], 0.0)
                            continue
                        ps = p2ps.tile([128, 64], F32)
                        nc.tensor.matmul(
                            out=ps[:], lhsT=hT2[0:64, i * 128:(i + 1) * 128],
                            rhs=nw1_sb[:], start=True, stop=False)
                        nc.tensor.matmul(
                            out=ps[:], lhsT=arena[:, w * 128:(w + 1) * 128],
                            rhs=nw2a_sb[:], start=False, stop=True)
                        if p2_level < 3:
                            nc.vector.tensor_copy(out=ost[:, i, :], in_=ps[:])
                        else:
                            nc.scalar.activation(
                                out=ost[:, i, :], in_=ps[:],
                                func=mybir.ActivationFunctionType.Relu)
                    nc.sync.dma_start(
                        out=out_d[g * GW * 128:(g + 1) * GW * 128, :].rearrange(
                            "(j p) f -> p j f", p=128),
                        in_=ost[:])

    nc.compile()
    return nc


# ---------------- host-side data prep ----------------

def _wrap16(a):
    x = np.ascontiguousarray(a.reshape(-1, 16).T)
    return np.tile(x, (8, 1))


def _wrap128(a):
    return np.ascontiguousarray(a.reshape(-1, 128).T)


def _wrap32x4(a):
    return np.ascontiguousarray(np.tile(a.reshape(-1, 32).T, (4, 1)))


def prep_inputs(cfg, h, edge_index, edge_w, edge_b, node_w, node_b):
    """Returns (in_maps). Sets cfg.SEG."""
    N = cfg.N
    row = np.asarray(edge_index[0])
    col = np.asarray(edge_index[1])
    h = np.asarray(h, dtype=np.float32)

    # hT augmented with ones row, padded to NP cols
    hTa = np.zeros((65, cfg.NP), np.float32)
    hTa[:64, :N] = h.T
    hTa[64, :] = 1.0
    hTab = hTa.astype(NP_BF16)

    waug = np.zeros((65, 128), np.float32)
    waug[:64, 0:64] = edge_w[:64]
    waug[:64, 64:128] = edge_w[64:]
    waug[64, 0:64] = edge_b
    waugb = waug.astype(NP_BF16)

    nw1 = np.ascontiguousarray(node_w[:64], dtype=np.float32)
    nw2a = np.concatenate([node_w[64:], node_b[None, :]], axis=0).astype(np.float32)

    iota = np.tile(np.arange(128, dtype=np.float32), (128, 1)).astype(NP_BF16)
    iota_st = np.add.outer((np.arange(128) // 32) * 32,
                           np.arange(32)).astype(NP_BF16)

    # per-core edge prep; SEG = global max segment length (uniform program)
    per_core = []
    maxc = 1
    for k in range(cfg.n_cores):
        base = k * cfg.NPC
        m = (row >= base) & (row < base + cfg.NPC)
        r = (row[m] - base).astype(np.int64)
        c = col[m].astype(np.int64)
        w = r >> 7
        cc = c // cfg.CHUNK
        seg_id = cc * cfg.W + w
        order = np.argsort(seg_id, kind="stable")
        r, c, seg_id = r[order], c[order], seg_id[order]
        counts = np.bincount(seg_id, minlength=cfg.C * cfg.W)
        if counts.size and r.size:
            maxc = max(maxc, int(counts.max()))
        per_core.append((r, c, seg_id, counts))
    SEG = int(math.ceil(maxc / 128.0)) * 128
    cfg.SEG = SEG
    EP = cfg.C * cfg.W * SEG

    in_maps = []
    for k in range(cfg.n_cores):
        r, c, seg_id, counts = per_core[k]
        starts = np.cumsum(counts) - counts
        intra = np.arange(r.size) - np.repeat(starts, counts)
        slots = seg_id * SEG + intra
        colidx = np.zeros(EP, np.int16)
        rl = np.full(EP, 255.0, NP_BF16)
        colidx[slots] = (c - (c // cfg.CHUNK) * cfg.CHUNK).astype(np.int16)
        rl[slots] = (r & 127).astype(NP_BF16)

        base = k * cfg.NPC
        hTown = np.ascontiguousarray(hTa[:, base:base + cfg.NPC])
        hTownb = np.ascontiguousarray(hTab[:, base:base + cfg.NPC])
        in_maps.append({
            "hTa": hTab,
            "hTownb": hTownb,
            "hTown": hTown,
            "waug": waugb,
            "nw1": nw1,
            "nw2a": nw2a,
            "iota": iota,
            "iota_st": iota_st,
            "colidx": _wrap16(colidx),
            "rl": _wrap128(rl),
            "rl4": _wrap32x4(rl),
        })
    return in_maps


def unshard_output(cfg, results):
    outs = [np.asarray(res["out"]) for res in results]
    full = np.concatenate(outs, axis=0)
    return np.ascontiguousarray(full[:cfg.N]).astype(np.float32)


# ---------------- entry point ----------------

def kernel(h, edge_index, edge_w, edge_b, node_w, node_b):
    from concourse.bass_utils import run_bass_kernel_spmd
    cfg = Cfg(n_nodes=100000, n_cores=8, spc=7)
    in_maps = prep_inputs(cfg, h, edge_index, edge_w, edge_b, node_w, node_b)
    nc = build_kernel(cfg)
    res = run_bass_kernel_spmd(nc, in_maps, core_ids=list(range(cfg.n_cores)))
    return unshard_output(cfg, res.results)


# revision 3
# speedup vs baseline: 1.3926x; 1.3926x over previous
"""Trainium2 Bass kernel for a GNN message-passing layer (GCL).

reference:
    m   = relu(concat(h[row], h[col]) @ edge_w + edge_b)       # [E, H]
    agg = segment_sum(m, row, N)                               # [N, H]
    out = relu(concat(h, agg) @ node_w + node_b)               # [N, H]

Strategy (8 cores, edge-parallel with node-range ownership, no collectives):
  * Precompute AB[n] = [h[n] @ Wtop + b | h[n] @ Wbot]  (bf16, DRAM table).
    Then m_e = relu(A[row_e] + B[col_e]).
  * B[col_e]: one dma_gather per edge (256B elems; descriptor count is the
    DMA bottleneck).  No row-side gather: A for the core's own 12544 nodes
    lives in SBUF ([128, W, 64] bf16, built by phase 0) and per-edge A is
    expanded on-chip with a one-hot matmul  psA = oh2^T @ A_window.
  * B is accumulated into the same PSUM tile with an identity matmul
    (psA += I^T @ B_gathered), so no DVE add is needed; relu reads PSUM.
  * Segment-sum via one-hot matmul: aggT += m2^T @ oh.
  * Both one-hots are host-precomputed fp8 indicator matrices (exact in
    fp8) streamed from DRAM -- building them on DVE with is_equal runs at
    1 elem/lane/cycle and was measured to cost ~1.8ms/core.
  * Node MLP with bias folded in via an appended ones-row (K=65 matmul).
Each core owns a contiguous 12544-node range; rows of its edges fall in that
range, so aggregation and the node MLP are fully local.
"""

import math
import numpy as np
import ml_dtypes

import concourse.bass as bass
import concourse.bacc as bacc
import concourse.tile as tile
from concourse import mybir
from concourse.tile import TileContext
from concourse.library_config import mlp as mlp_library

BF16 = mybir.dt.bfloat16
F32 = mybir.dt.float32
I16 = mybir.dt.int16
FP8 = mybir.dt.float8e4
NP_BF16 = ml_dtypes.bfloat16
NP_FP8 = ml_dtypes.float8_e4m3fn


class Cfg:
    def __init__(self, n_nodes, n_cores=8, spc=7, table_f32=False):
        self.n_swdge_queues = 4   # parallel SWDGE queues
        self.N = n_nodes
        self.n_cores = n_cores
        self.NPC = int(math.ceil(n_nodes / n_cores / 128)) * 128
        self.NP = self.NPC * n_cores
        self.W = self.NPC // 128          # windows per core
        self.C = 4                        # col chunks (int16 gather idx limit)
        assert self.NP % self.C == 0
        self.CHUNK = self.NP // self.C
        assert self.CHUNK <= 32767, "int16 gather index limit"
        # segments (windows) per gather call; must divide W
        self.SPC = spc
        assert self.W % self.SPC == 0
        self.CALLS_PER_CHUNK = self.W // self.SPC
        # idx loads cover IDX_CALLS gather calls each
        self.IDX_CALLS = self.CALLS_PER_CHUNK // 2 if self.CALLS_PER_CHUNK % 2 == 0 else self.CALLS_PER_CHUNK
        self.SEG = None  # set from data

    def stripe(self, total):
        for cand in (8192, 6272, 4096, 3136, 2048, 1792, 1568, 1024, 896, 784, 512, 448, 256, 128):
            if cand <= total and total % cand == 0:
                return cand
        raise AssertionError(total)


def build_kernel(cfg, phases=(0, 1, 2), p1_level=4, p2_level=3):
    """Build the single-core SPMD program. Returns nc.
    p1_level: 1=gathers only, 2=+onehot loads, 3=+matmuls/relu, 4=full."""
    SEG = cfg.SEG
    assert SEG is not None and SEG % 128 == 0
    EP = cfg.C * cfg.W * SEG               # padded edges per core
    NCALL = cfg.SPC * SEG                  # idxs per gather call
    JPC = NCALL // 128                     # 128-chunks per call
    JPS = SEG // 128                       # 128-chunks per segment
    # >64 descriptors/engine in one packet wedges the device; the per-engine
    # descriptor count is NCALL//16 + 1.
    SINGLE_PACKET = (NCALL // 16 + 1) <= 64

    NSWQ = getattr(cfg, "n_swdge_queues", 1)
    nc = bacc.Bacc("TRN2", target_bir_lowering=False, debug=False,
                   num_swdge_queues=NSWQ)

    # ---- DRAM I/O ----
    hTa_d = nc.dram_tensor("hTa", [65, cfg.NP], BF16, kind="ExternalInput")
    hTownb_d = nc.dram_tensor("hTownb", [65, cfg.NPC], BF16, kind="ExternalInput")
    hTown_d = nc.dram_tensor("hTown", [65, cfg.NPC], F32, kind="ExternalInput")
    waug_d = nc.dram_tensor("waug", [65, 128], BF16, kind="ExternalInput")
    nw1_d = nc.dram_tensor("nw1", [64, 64], F32, kind="ExternalInput")
    nw2a_d = nc.dram_tensor("nw2a", [65, 64], F32, kind="ExternalInput")
    ident_d = nc.dram_tensor("ident", [128, 128], BF16, kind="ExternalInput")
    colidx_d = nc.dram_tensor("colidx", [128, EP // 16], I16, kind="ExternalInput")
    oht_d = nc.dram_tensor("oht", [128, EP], FP8, kind="ExternalInput")
    oh2t_d = nc.dram_tensor("oh2t", [128, EP], FP8, kind="ExternalInput")
    AB_ds = [nc.dram_tensor(f"AB{c}", [cfg.CHUNK, 128], BF16)
             for c in range(cfg.C)]
    out_d = nc.dram_tensor("out", [cfg.NPC, 64], F32, kind="ExternalOutput")

    with TileContext(nc) as tc:
        nc.gpsimd.load_library(mlp_library)

        with tc.tile_pool(name="const", bufs=1) as cpool:
            waug_sb = cpool.tile([65, 128], BF16)
            nc.sync.dma_start(out=waug_sb[:], in_=waug_d[:])
            ident_sb = cpool.tile([128, 128], BF16)
            nc.sync.dma_start(out=ident_sb[:], in_=ident_d[:])
            nw1_sb = cpool.tile([64, 64], F32)
            nc.sync.dma_start(out=nw1_sb[:], in_=nw1_d[:])
            nw2a_sb = cpool.tile([65, 64], F32)
            nc.sync.dma_start(out=nw2a_sb[:], in_=nw2a_d[:])

            # aggT arena [65, NPC]: rows 0:64 = aggT, row 64 = ones (bias row)
            arena = cpool.tile([65, cfg.NPC], F32)
            nc.vector.memset(arena[64:65, :], 1.0)

            # A table for own nodes, SBUF-resident: [128, W, 64] bf16
            aown_sb = cpool.tile([128, cfg.W, 64], BF16)

            # ---- Phase 0: build AB table (all NP nodes) + Aown (SBUF) ----
            if 0 in phases:
                with tc.tile_pool(name="p0a", bufs=1) as p0a, \
                     tc.tile_pool(name="p0aps", bufs=4, space="PSUM") as p0aps:
                    htb = p0a.tile([65, cfg.NPC], BF16)
                    nc.sync.dma_start(out=htb[:], in_=hTownb_d[:])
                    for w in range(cfg.W):
                        ps = p0aps.tile([128, 64], F32)
                        nc.tensor.matmul(
                            out=ps[:], lhsT=htb[:, w * 128:(w + 1) * 128],
                            rhs=waug_sb[:, 0:64], start=True, stop=True)
                        nc.vector.tensor_copy(out=aown_sb[:, w, :], in_=ps[:])

                SN = cfg.stripe(cfg.CHUNK)
                JT = SN // 128
                with tc.tile_pool(name="p0", bufs=2) as p0, \
                     tc.tile_pool(name="p0ps", bufs=4, space="PSUM") as p0ps:
                    for s in range(cfg.NP // SN):
                        hstripe = p0.tile([65, SN], BF16, tag="hstripe")
                        nc.sync.dma_start(
                            out=hstripe[:], in_=hTa_d[:, s * SN:(s + 1) * SN])
                        abst = p0.tile([128, JT, 128], BF16, tag="abst")
                        for j in range(JT):
                            ps = p0ps.tile([128, 128], F32)
                            nc.tensor.matmul(
                                out=ps[:], lhsT=hstripe[:, j * 128:(j + 1) * 128],
                                rhs=waug_sb[:], start=True, stop=True)
                            nc.vector.tensor_copy(out=abst[:, j, :], in_=ps[:])
                        n0 = s * SN
                        dst_d, off = AB_ds[n0 // cfg.CHUNK], n0 % cfg.CHUNK
                        nc.sync.dma_start(
                            out=dst_d[off:off + SN, :].rearrange(
                                "(j p) f -> p j f", p=128),
                            in_=abst[:])

            # ---- Phase 1: gather + edge MLP + one-hot aggregation ----
            if 1 in phases:
                with tc.tile_pool(name="idxp", bufs=2) as idxp, \
                     tc.tile_pool(name="gath", bufs=3) as gathp, \
                     tc.tile_pool(name="ohtp", bufs=2) as ohtp, \
                     tc.tile_pool(name="oh2tp", bufs=2) as oh2tp, \
                     tc.tile_pool(name="mp", bufs=3) as mp, \
                     tc.tile_pool(name="psA", bufs=4, space="PSUM") as psAp, \
                     tc.tile_pool(name="p1ps", bufs=4, space="PSUM") as p1ps:
                    IC = cfg.IDX_CALLS
                    ILEN = IC * NCALL // 16      # idx cols per load
                    for c in range(cfg.C):
                        for g in range(cfg.CALLS_PER_CHUNK // IC):
                            goff = (c * cfg.CALLS_PER_CHUNK + g * IC) * NCALL // 16
                            cidx = idxp.tile([128, ILEN], I16, tag="cidx")
                            nc.sync.dma_start(
                                out=cidx[:], in_=colidx_d[:, goff:goff + ILEN])
                            for cl in range(IC):
                                call = (c * cfg.CALLS_PER_CHUNK + g * IC + cl)
                                colg = gathp.tile([128, JPC, 128], BF16, tag="g")
                                nc.gpsimd.dma_gather(
                                    colg[:], AB_ds[c][:],
                                    cidx[:, cl * (NCALL // 16):(cl + 1) * (NCALL // 16)],
                                    NCALL, NCALL, 128, single_packet=SINGLE_PACKET,
                                    queue_num=call % NSWQ)
                                if p1_level < 2:
                                    continue
                                e0 = call * NCALL
                                oht = ohtp.tile([128, JPC, 128], FP8, tag="oht")
                                nc.sync.dma_start(
                                    out=oht[:],
                                    in_=oht_d[:, e0:e0 + NCALL].rearrange(
                                        "p (a b) -> p a b", b=128))
                                oh2t = oh2tp.tile([128, JPC, 128], FP8, tag="oh2t")
                                nc.sync.dma_start(
                                    out=oh2t[:],
                                    in_=oh2t_d[:, e0:e0 + NCALL].rearrange(
                                        "p (a b) -> p a b", b=128))
                                for s in range(cfg.SPC if p1_level >= 3 else 0):
                                    w = (call % cfg.CALLS_PER_CHUNK) * cfg.SPC + s
                                    gseg = call * cfg.SPC + s
                                    j0 = s * JPS
                                    m2 = mp.tile([128, JPS, 64], BF16, tag="m2")
                                    for jj in range(0, JPS, 3):
                                        gs = min(3, JPS - jj)
                                        pa = psAp.tile([128, gs, 64], F32)
                                        for j in range(gs):
                                            nc.tensor.matmul(
                                                out=pa[:, j, :],
                                                lhsT=oh2t[:, j0 + jj + j, :],
                                                rhs=aown_sb[:, w, :],
                                                start=True, stop=False)
                                            nc.tensor.matmul(
                                                out=pa[:, j, :],
                                                lhsT=ident_sb[:],
                                                rhs=colg[:, j0 + jj + j, 64:128],
                                                start=False, stop=True)
                                        # relu, alternating Act/DVE by segment
                                        if gseg % 2 == 0:
                                            nc.scalar.activation(
                                                out=m2[:, jj:jj + gs, :], in_=pa[:],
                                                func=mybir.ActivationFunctionType.Relu)
                                        else:
                                            nc.vector.tensor_scalar_max(
                                                m2[:, jj:jj + gs, :], pa[:], 0.0)
                                    if p1_level < 4:
                                        continue
                                    ps = p1ps.tile([64, 128], F32)
                                    for j in range(JPS):
                                        nc.tensor.matmul(
                                            out=ps[:], lhsT=m2[:, j, :],
                                            rhs=oht[:, j0 + j, :],
                                            start=(j == 0), stop=(j == JPS - 1))
                                    dst = arena[0:64, w * 128:(w + 1) * 128]
                                    if c == 0:
                                        nc.vector.tensor_copy(out=dst, in_=ps[:])
                                    else:
                                        nc.vector.tensor_tensor(
                                            out=dst, in0=dst, in1=ps[:],
                                            op=mybir.AluOpType.add)

            # ---- Phase 2: node MLP ----
            if 2 in phases:
              with tc.tile_pool(name="p2", bufs=2) as p2, \
                 tc.tile_pool(name="p2ps", bufs=4, space="PSUM") as p2ps:
                GW = cfg.SPC                  # windows per output group
                for g in range(cfg.W // GW):
                    hT2 = p2.tile([65, GW * 128], F32, tag="hT2")
                    nc.sync.dma_start(
                        out=hT2[:],
                        in_=hTown_d[:, g * GW * 128:(g + 1) * GW * 128])
                    ost = p2.tile([128, GW, 64], F32, tag="ost")
                    for i in range(GW):
                        w = g * GW + i
                        if p2_level < 2:
                            nc.vector.memset(ost[:, i, :], 0.0)
                            continue
                        ps = p2ps.tile([128, 64], F32)
                        nc.tensor.matmul(
                            out=ps[:], lhsT=hT2[0:64, i * 128:(i + 1) * 128],
                            rhs=nw1_sb[:], start=True, stop=False)
                        nc.tensor.matmul(
                            out=ps[:], lhsT=arena[:, w * 128:(w + 1) * 128],
                            rhs=nw2a_sb[:], start=False, stop=True)
                        if p2_level < 3:
                            nc.vector.tensor_copy(out=ost[:, i, :], in_=ps[:])
                        else:
                            nc.scalar.activation(
                                out=ost[:, i, :], in_=ps[:],
                                func=mybir.ActivationFunctionType.Relu)
                    nc.sync.dma_start(
                        out=out_d[g * GW * 128:(g + 1) * GW * 128, :].rearrange(
                            "(j p) f -> p j f", p=128),
                        in_=ost[:])

    nc.compile()
    return nc


# ---------------- host-side data prep ----------------

def _wrap16(a):
    x = np.ascontiguousarray(a.reshape(-1, 16).T)
    return np.tile(x, (8, 1))


def prep_inputs(cfg, h, edge_index, edge_w, edge_b, node_w, node_b):
    """Returns in_maps. Sets cfg.SEG."""
    N = cfg.N
    row = np.asarray(edge_index[0])
    col = np.asarray(edge_index[1])
    h = np.asarray(h, dtype=np.float32)

    # hT augmented with ones row, padded to NP cols
    hTa = np.zeros((65, cfg.NP), np.float32)
    hTa[:64, :N] = h.T
    hTa[64, :] = 1.0
    hTab = hTa.astype(NP_BF16)

    waug = np.zeros((65, 128), np.float32)
    waug[:64, 0:64] = edge_w[:64]
    waug[:64, 64:128] = edge_w[64:]
    waug[64, 0:64] = edge_b
    waugb = waug.astype(NP_BF16)

    nw1 = np.ascontiguousarray(node_w[:64], dtype=np.float32)
    nw2a = np.concatenate([node_w[64:], node_b[None, :]], axis=0).astype(np.float32)

    ident = np.eye(128, dtype=np.float32).astype(NP_BF16)

    # per-core edge prep; SEG = global max segment length (uniform program)
    per_core = []
    maxc = 1
    for k in range(cfg.n_cores):
        base = k * cfg.NPC
        m = (row >= base) & (row < base + cfg.NPC)
        r = (row[m] - base).astype(np.int64)
        c = col[m].astype(np.int64)
        w = r >> 7
        cc = c // cfg.CHUNK
        seg_id = cc * cfg.W + w
        order = np.argsort(seg_id, kind="stable")
        r, c, seg_id = r[order], c[order], seg_id[order]
        counts = np.bincount(seg_id, minlength=cfg.C * cfg.W)
        if counts.size and r.size:
            maxc = max(maxc, int(counts.max()))
        per_core.append((r, c, seg_id, counts))
    SEG = int(math.ceil(maxc / 128.0)) * 128
    cfg.SEG = SEG
    EP = cfg.C * cfg.W * SEG

    nvals = np.arange(128, dtype=np.int16)
    in_maps = []
    for k in range(cfg.n_cores):
        r, c, seg_id, counts = per_core[k]
        starts = np.cumsum(counts) - counts
        intra = np.arange(r.size) - np.repeat(starts, counts)
        slots = seg_id * SEG + intra
        colidx = np.zeros(EP, np.int16)
        rl = np.full(EP, 255, np.int16)
        colidx[slots] = (c - (c // cfg.CHUNK) * cfg.CHUNK).astype(np.int16)
        rl[slots] = (r & 127).astype(np.int16)

        # fp8 one-hot indicator matrices (exact in fp8)
        ohf = (rl[:, None] == nvals[None, :]).astype(NP_FP8)   # [EP, 128]
        A3 = ohf.reshape(EP // 128, 128, 128)                  # [j, e', n]
        oht = np.ascontiguousarray(
            A3.transpose(1, 0, 2).reshape(128, EP))            # [p=e', j, n]
        oh2t = np.ascontiguousarray(
            A3.transpose(2, 0, 1).reshape(128, EP))            # [p=n, j, e']

        base = k * cfg.NPC
        hTown = np.ascontiguousarray(hTa[:, base:base + cfg.NPC])
        hTownb = np.ascontiguousarray(hTab[:, base:base + cfg.NPC])
        in_maps.append({
            "hTa": hTab,
            "hTownb": hTownb,
            "hTown": hTown,
            "waug": waugb,
            "nw1": nw1,
            "nw2a": nw2a,
            "ident": ident,
            "colidx": _wrap16(colidx),
            "oht": oht,
            "oh2t": oh2t,
        })
    return in_maps


def unshard_output(cfg, results):
    outs = [np.asarray(res["out"]) for res in results]
    full = np.concatenate(outs, axis=0)
    return np.ascontiguousarray(full[:cfg.N]).astype(np.float32)


# ---------------- entry point ----------------

def kernel(h, edge_index, edge_w, edge_b, node_w, node_b):
    from concourse.bass_utils import run_bass_kernel_spmd
    cfg = Cfg(n_nodes=100000, n_cores=8, spc=7)
    in_maps = prep_inputs(cfg, h, edge_index, edge_w, edge_b, node_w, node_b)
    nc = build_kernel(cfg)
    res = run_bass_kernel_spmd(nc, in_maps, core_ids=list(range(cfg.n_cores)))
    return unshard_output(cfg, res.results)


# revision 6
# speedup vs baseline: 1.7333x; 1.2446x over previous
"""Trainium2 Bass kernel for a GNN message-passing layer (GCL).

reference:
    m   = relu(concat(h[row], h[col]) @ edge_w + edge_b)       # [E, H]
    agg = segment_sum(m, row, N)                               # [N, H]
    out = relu(concat(h, agg) @ node_w + node_b)               # [N, H]

Strategy (8 cores, edge-parallel with node-range ownership, no collectives):
  * Precompute AB[n] = [h[n] @ Wtop + b | h[n] @ Wbot]  (bf16, DRAM table).
    Then m_e = relu(A[row_e] + B[col_e]).
  * B[col_e]: one dma_gather per edge (256B elems; descriptor count is the
    DMA bottleneck).  No row-side gather: A for the core's own 12544 nodes
    lives in SBUF ([128, W, 64] bf16, built by phase 0) and per-edge A is
    expanded on-chip with a one-hot matmul  psA = oh2^T @ A_window.
  * B is accumulated into the same PSUM tile with an identity matmul
    (psA += I^T @ B_gathered), so no DVE add is needed; relu reads PSUM.
  * Segment-sum via one-hot matmul: aggT += m2^T @ oh.
  * Both one-hots are host-precomputed fp8 indicator matrices (exact in
    fp8) streamed from DRAM -- building them on DVE with is_equal runs at
    1 elem/lane/cycle and was measured to cost ~1.8ms/core.
  * Node MLP with bias folded in via an appended ones-row (K=65 matmul).
Each core owns a contiguous 12544-node range; rows of its edges fall in that
range, so aggregation and the node MLP are fully local.
"""

import math
import numpy as np
import ml_dtypes

import concourse.bass as bass
import concourse.bacc as bacc
import concourse.tile as tile
from concourse import mybir
from concourse.tile import TileContext
from concourse.library_config import mlp as mlp_library

BF16 = mybir.dt.bfloat16
F32 = mybir.dt.float32
I16 = mybir.dt.int16
FP8 = mybir.dt.float8e4
NP_BF16 = ml_dtypes.bfloat16
NP_FP8 = ml_dtypes.float8_e4m3fn


class Cfg:
    def __init__(self, n_nodes, n_cores=8, spc=7, table_f32=False):
        self.n_swdge_queues = 4   # parallel SWDGE queues
        self.N = n_nodes
        self.n_cores = n_cores
        self.NPC = int(math.ceil(n_nodes / n_cores / 128)) * 128
        self.NP = self.NPC * n_cores
        self.W = self.NPC // 128          # windows per core
        self.C = 4                        # col chunks (int16 gather idx limit)
        assert self.NP % self.C == 0
        self.CHUNK = self.NP // self.C
        assert self.CHUNK <= 32767, "int16 gather index limit"
        # segments (windows) per gather call; must divide W
        self.SPC = spc
        assert self.W % self.SPC == 0
        self.CALLS_PER_CHUNK = self.W // self.SPC
        # idx loads cover IDX_CALLS gather calls each
        self.IDX_CALLS = self.CALLS_PER_CHUNK // 2 if self.CALLS_PER_CHUNK % 2 == 0 else self.CALLS_PER_CHUNK
        self.SEG = None  # set from data

    def stripe(self, total):
        for cand in (8192, 6272, 4096, 3136, 2048, 1792, 1568, 1024, 896, 784, 512, 448, 256, 128):
            if cand <= total and total % cand == 0:
                return cand
        raise AssertionError(total)


def build_kernel(cfg, phases=(0, 1, 2), p1_level=4, p2_level=3):
    """Build the single-core SPMD program. Returns nc.
    p1_level: 1=gathers only, 2=+onehot loads, 3=+matmuls/relu, 4=full."""
    SEG = cfg.SEG
    assert SEG is not None and SEG % 128 == 0
    EP = cfg.C * cfg.W * SEG               # padded edges per core
    NCALL = cfg.SPC * SEG                  # idxs per gather call
    JPC = NCALL // 128                     # 128-chunks per call
    JPS = SEG // 128                       # 128-chunks per segment
    # >64 descriptors/engine in one packet wedges the device; the per-engine
    # descriptor count is NCALL//16 + 1.
    SINGLE_PACKET = (NCALL // 16 + 1) <= 64

    NSWQ = getattr(cfg, "n_swdge_queues", 1)
    nc = bacc.Bacc("TRN2", target_bir_lowering=False, debug=False,
                   num_swdge_queues=NSWQ)

    # ---- DRAM I/O ----
    hTa_d = nc.dram_tensor("hTa", [65, cfg.NP], BF16, kind="ExternalInput")
    hTownb_d = nc.dram_tensor("hTownb", [65, cfg.NPC], BF16, kind="ExternalInput")
    hTown_d = nc.dram_tensor("hTown", [65, cfg.NPC], F32, kind="ExternalInput")
    waug_d = nc.dram_tensor("waug", [65, 128], BF16, kind="ExternalInput")
    nw1_d = nc.dram_tensor("nw1", [64, 64], F32, kind="ExternalInput")
    nw2a_d = nc.dram_tensor("nw2a", [65, 64], F32, kind="ExternalInput")
    ident_d = nc.dram_tensor("ident", [128, 128], BF16, kind="ExternalInput")
    colidx_d = nc.dram_tensor("colidx", [128, EP // 16], I16, kind="ExternalInput")
    oht_d = nc.dram_tensor("oht", [128, EP], FP8, kind="ExternalInput")
    oh2t_d = nc.dram_tensor("oh2t", [128, EP], FP8, kind="ExternalInput")
    AB_ds = [nc.dram_tensor(f"AB{c}", [cfg.CHUNK, 128], BF16)
             for c in range(cfg.C)]
    out_d = nc.dram_tensor("out", [cfg.NPC, 64], F32, kind="ExternalOutput")

    with TileContext(nc) as tc:
        nc.gpsimd.load_library(mlp_library)

        with tc.tile_pool(name="const", bufs=1) as cpool:
            waug_sb = cpool.tile([65, 128], BF16)
            nc.sync.dma_start(out=waug_sb[:], in_=waug_d[:])
            ident_sb = cpool.tile([128, 128], BF16)
            nc.sync.dma_start(out=ident_sb[:], in_=ident_d[:])
            nw1_sb = cpool.tile([64, 64], F32)
            nc.sync.dma_start(out=nw1_sb[:], in_=nw1_d[:])
            nw2a_sb = cpool.tile([65, 64], F32)
            nc.sync.dma_start(out=nw2a_sb[:], in_=nw2a_d[:])

            # aggT arena [65, NPC]: rows 0:64 = aggT, row 64 = ones (bias row)
            arena = cpool.tile([65, cfg.NPC], F32)
            nc.vector.memset(arena[64:65, :], 1.0)

            # A table for own nodes, SBUF-resident: [128, W, 64] bf16
            aown_sb = cpool.tile([128, cfg.W, 64], BF16)

            # ---- Phase 0: build AB table (all NP nodes) + Aown (SBUF) ----
            if 0 in phases:
                with tc.tile_pool(name="p0a", bufs=1) as p0a, \
                     tc.tile_pool(name="p0aps", bufs=4, space="PSUM") as p0aps:
                    htb = p0a.tile([65, cfg.NPC], BF16)
                    nc.sync.dma_start(out=htb[:], in_=hTownb_d[:])
                    for w in range(cfg.W):
                        ps = p0aps.tile([128, 64], F32)
                        nc.tensor.matmul(
                            out=ps[:], lhsT=htb[:, w * 128:(w + 1) * 128],
                            rhs=waug_sb[:, 0:64], start=True, stop=True)
                        nc.vector.tensor_copy(out=aown_sb[:, w, :], in_=ps[:])

                SN = cfg.stripe(cfg.CHUNK)
                JT = SN // 128
                with tc.tile_pool(name="p0", bufs=2) as p0, \
                     tc.tile_pool(name="p0ps", bufs=4, space="PSUM") as p0ps:
                    for s in range(cfg.NP // SN):
                        hstripe = p0.tile([65, SN], BF16, tag="hstripe")
                        nc.sync.dma_start(
                            out=hstripe[:], in_=hTa_d[:, s * SN:(s + 1) * SN])
                        abst = p0.tile([128, JT, 128], BF16, tag="abst")
                        for j in range(JT):
                            ps = p0ps.tile([128, 128], F32)
                            nc.tensor.matmul(
                                out=ps[:], lhsT=hstripe[:, j * 128:(j + 1) * 128],
                                rhs=waug_sb[:], start=True, stop=True)
                            nc.vector.tensor_copy(out=abst[:, j, :], in_=ps[:])
                        n0 = s * SN
                        dst_d, off = AB_ds[n0 // cfg.CHUNK], n0 % cfg.CHUNK
                        nc.sync.dma_start(
                            out=dst_d[off:off + SN, :].rearrange(
                                "(j p) f -> p j f", p=128),
                            in_=abst[:])

            # ---- Phase 1 + 2, interleaved by window group ----
            # Edge slots are ordered (window-group, chunk, window, intra) so
            # a group's 4 chunk-calls complete together and its node MLP
            # (phase 2) runs inline, overlapped with later groups' phase 1.
            if 1 in phases:
                with tc.tile_pool(name="idxp", bufs=2) as idxp, \
                     tc.tile_pool(name="gath", bufs=3) as gathp, \
                     tc.tile_pool(name="ohtp", bufs=2) as ohtp, \
                     tc.tile_pool(name="oh2tp", bufs=2) as oh2tp, \
                     tc.tile_pool(name="mp", bufs=3) as mp, \
                     tc.tile_pool(name="p2", bufs=2) as p2, \
                     tc.tile_pool(name="psA", bufs=3, space="PSUM") as psAp, \
                     tc.tile_pool(name="p1ps", bufs=3, space="PSUM") as p1ps, \
                     tc.tile_pool(name="p2ps", bufs=2, space="PSUM") as p2ps:
                    ILEN = cfg.C * NCALL // 16      # idx cols per group
                    for g in range(cfg.W // cfg.SPC):
                        cidx = idxp.tile([128, ILEN], I16, tag="cidx")
                        nc.sync.dma_start(
                            out=cidx[:],
                            in_=colidx_d[:, g * ILEN:(g + 1) * ILEN])
                        if 2 in phases:
                            hT2 = p2.tile([65, cfg.SPC * 128], F32, tag="hT2")
                            nc.sync.dma_start(
                                out=hT2[:],
                                in_=hTown_d[:, g * cfg.SPC * 128:(g + 1) * cfg.SPC * 128])
                        for c in range(cfg.C):
                            call = g * cfg.C + c
                            colg = gathp.tile([128, JPC, 128], BF16, tag="g")
                            nc.gpsimd.dma_gather(
                                colg[:], AB_ds[c][:],
                                cidx[:, c * (NCALL // 16):(c + 1) * (NCALL // 16)],
                                NCALL, NCALL, 128, single_packet=SINGLE_PACKET,
                                queue_num=call % NSWQ)
                            if p1_level < 2:
                                continue
                            e0 = call * NCALL
                            oht = ohtp.tile([128, JPC, 128], FP8, tag="oht")
                            nc.sync.dma_start(
                                out=oht[:],
                                in_=oht_d[:, e0:e0 + NCALL].rearrange(
                                    "p (a b) -> p a b", b=128))
                            oh2t = oh2tp.tile([128, JPC, 128], FP8, tag="oh2t")
                            nc.sync.dma_start(
                                out=oh2t[:],
                                in_=oh2t_d[:, e0:e0 + NCALL].rearrange(
                                    "p (a b) -> p a b", b=128))
                            for s in range(cfg.SPC if p1_level >= 3 else 0):
                                w = g * cfg.SPC + s
                                gseg = call * cfg.SPC + s
                                j0 = s * JPS
                                m2 = mp.tile([128, JPS, 64], BF16, tag="m2")
                                for jj in range(0, JPS, 3):
                                    gs = min(3, JPS - jj)
                                    pa = psAp.tile([128, gs, 64], F32)
                                    for j in range(gs):
                                        nc.tensor.matmul(
                                            out=pa[:, j, :],
                                            lhsT=oh2t[:, j0 + jj + j, :],
                                            rhs=aown_sb[:, w, :],
                                            start=True, stop=False)
                                        nc.tensor.matmul(
                                            out=pa[:, j, :],
                                            lhsT=ident_sb[:],
                                            rhs=colg[:, j0 + jj + j, 64:128],
                                            start=False, stop=True)
                                    # relu, alternating Act/DVE by segment
                                    if gseg % 2 == 0:
                                        nc.scalar.activation(
                                            out=m2[:, jj:jj + gs, :], in_=pa[:],
                                            func=mybir.ActivationFunctionType.Relu)
                                    else:
                                        nc.vector.tensor_scalar_max(
                                            m2[:, jj:jj + gs, :], pa[:], 0.0)
                                if p1_level < 4:
                                    continue
                                ps = p1ps.tile([64, 128], F32)
                                for j in range(JPS):
                                    nc.tensor.matmul(
                                        out=ps[:], lhsT=m2[:, j, :],
                                        rhs=oht[:, j0 + j, :],
                                        start=(j == 0), stop=(j == JPS - 1))
                                dst = arena[0:64, w * 128:(w + 1) * 128]
                                if c == 0:
                                    nc.vector.tensor_copy(out=dst, in_=ps[:])
                                else:
                                    nc.vector.tensor_tensor(
                                        out=dst, in0=dst, in1=ps[:],
                                        op=mybir.AluOpType.add)
                        # node MLP for this window group
                        if 2 in phases and p1_level >= 4:
                            ost = p2.tile([128, cfg.SPC, 64], F32, tag="ost")
                            for i in range(cfg.SPC):
                                w = g * cfg.SPC + i
                                if p2_level < 2:
                                    nc.vector.memset(ost[:, i, :], 0.0)
                                    continue
                                ps = p2ps.tile([128, 64], F32)
                                nc.tensor.matmul(
                                    out=ps[:], lhsT=hT2[0:64, i * 128:(i + 1) * 128],
                                    rhs=nw1_sb[:], start=True, stop=False)
                                nc.tensor.matmul(
                                    out=ps[:], lhsT=arena[:, w * 128:(w + 1) * 128],
                                    rhs=nw2a_sb[:], start=False, stop=True)
                                if p2_level < 3:
                                    nc.vector.tensor_copy(out=ost[:, i, :], in_=ps[:])
                                else:
                                    nc.scalar.activation(
                                        out=ost[:, i, :], in_=ps[:],
                                        func=mybir.ActivationFunctionType.Relu)
                            nc.sync.dma_start(
                                out=out_d[g * cfg.SPC * 128:(g + 1) * cfg.SPC * 128, :]
                                    .rearrange("(j p) f -> p j f", p=128),
                                in_=ost[:])

    nc.compile()
    return nc


# ---------------- host-side data prep ----------------

def _wrap16(a):
    x = np.ascontiguousarray(a.reshape(-1, 16).T)
    return np.tile(x, (8, 1))


def prep_inputs(cfg, h, edge_index, edge_w, edge_b, node_w, node_b):
    """Returns in_maps. Sets cfg.SEG."""
    N = cfg.N
    row = np.asarray(edge_index[0])
    col = np.asarray(edge_index[1])
    h = np.asarray(h, dtype=np.float32)

    # hT augmented with ones row, padded to NP cols
    hTa = np.zeros((65, cfg.NP), np.float32)
    hTa[:64, :N] = h.T
    hTa[64, :] = 1.0
    hTab = hTa.astype(NP_BF16)

    waug = np.zeros((65, 128), np.float32)
    waug[:64, 0:64] = edge_w[:64]
    waug[:64, 64:128] = edge_w[64:]
    waug[64, 0:64] = edge_b
    waugb = waug.astype(NP_BF16)

    nw1 = np.ascontiguousarray(node_w[:64], dtype=np.float32)
    nw2a = np.concatenate([node_w[64:], node_b[None, :]], axis=0).astype(np.float32)

    ident = np.eye(128, dtype=np.float32).astype(NP_BF16)

    # per-core edge prep; SEG = global max segment length (uniform program)
    per_core = []
    maxc = 1
    for k in range(cfg.n_cores):
        base = k * cfg.NPC
        m = (row >= base) & (row < base + cfg.NPC)
        r = (row[m] - base).astype(np.int64)
        c = col[m].astype(np.int64)
        w = r >> 7
        cc = c // cfg.CHUNK
        # segment order: (window-group, chunk, window-within-group)
        seg_id = (w // cfg.SPC) * (cfg.C * cfg.SPC) + cc * cfg.SPC + (w % cfg.SPC)
        order = np.argsort(seg_id, kind="stable")
        r, c, seg_id = r[order], c[order], seg_id[order]
        counts = np.bincount(seg_id, minlength=cfg.C * cfg.W)
        if counts.size and r.size:
            maxc = max(maxc, int(counts.max()))
        per_core.append((r, c, seg_id, counts))
    SEG = int(math.ceil(maxc / 128.0)) * 128
    cfg.SEG = SEG
    EP = cfg.C * cfg.W * SEG

    nvals = np.arange(128, dtype=np.int16)
    in_maps = []
    for k in range(cfg.n_cores):
        r, c, seg_id, counts = per_core[k]
        starts = np.cumsum(counts) - counts
        intra = np.arange(r.size) - np.repeat(starts, counts)
        slots = seg_id * SEG + intra
        colidx = np.zeros(EP, np.int16)
        rl = np.full(EP, 255, np.int16)
        colidx[slots] = (c - (c // cfg.CHUNK) * cfg.CHUNK).astype(np.int16)
        rl[slots] = (r & 127).astype(np.int16)

        # fp8 one-hot indicator matrices (exact in fp8)
        ohf = (rl[:, None] == nvals[None, :]).astype(NP_FP8)   # [EP, 128]
        A3 = ohf.reshape(EP // 128, 128, 128)                  # [j, e', n]
        oht = np.ascontiguousarray(
            A3.transpose(1, 0, 2).reshape(128, EP))            # [p=e', j, n]
        oh2t = np.ascontiguousarray(
            A3.transpose(2, 0, 1).reshape(128, EP))            # [p=n, j, e']

        base = k * cfg.NPC
        hTown = np.ascontiguousarray(hTa[:, base:base + cfg.NPC])
        hTownb = np.ascontiguousarray(hTab[:, base:base + cfg.NPC])
        in_maps.append({
            "hTa": hTab,
            "hTownb": hTownb,
            "hTown": hTown,
            "waug": waugb,
            "nw1": nw1,
            "nw2a": nw2a,
            "ident": ident,
            "colidx": _wrap16(colidx),
            "oht": oht,
            "oh2t": oh2t,
        })
    return in_maps


def unshard_output(cfg, results):
    outs = [np.asarray(res["out"]) for res in results]
    full = np.concatenate(outs, axis=0)
    return np.ascontiguousarray(full[:cfg.N]).astype(np.float32)


# ---------------- entry point ----------------

def kernel(h, edge_index, edge_w, edge_b, node_w, node_b):
    from concourse.bass_utils import run_bass_kernel_spmd
    cfg = Cfg(n_nodes=100000, n_cores=8, spc=7)
    in_maps = prep_inputs(cfg, h, edge_index, edge_w, edge_b, node_w, node_b)
    nc = build_kernel(cfg)
    res = run_bass_kernel_spmd(nc, in_maps, core_ids=list(range(cfg.n_cores)))
    return unshard_output(cfg, res.results)


# revision 11
# speedup vs baseline: 6.4062x; 3.6960x over previous
"""Trainium2 Bass kernel for a GNN message-passing layer (GCL).

reference:
    m   = relu(concat(h[row], h[col]) @ edge_w + edge_b)       # [E, H]
    agg = segment_sum(m, row, N)                               # [N, H]
    out = relu(concat(h, agg) @ node_w + node_b)               # [N, H]

Strategy (8 cores, edge-parallel with node-range ownership, no collectives):
  * Precompute AB[n] = [h[n] @ Wtop + b | h[n] @ Wbot]  (bf16, DRAM table).
    Then m_e = relu(A[row_e] + B[col_e]).
  * B[col_e]: one dma_gather per edge (256B elems; descriptor count is the
    DMA bottleneck).  No row-side gather: A for the core's own 12544 nodes
    lives in SBUF ([128, W, 64] bf16, built by phase 0) and per-edge A is
    expanded on-chip with a one-hot matmul  psA = oh2^T @ A_window.
  * B is accumulated into the same PSUM tile with an identity matmul
    (psA += I^T @ B_gathered), so no DVE add is needed; relu reads PSUM.
  * Segment-sum via one-hot matmul: aggT += m2^T @ oh.
  * Both one-hots are host-precomputed fp8 indicator matrices (exact in
    fp8) streamed from DRAM -- building them on DVE with is_equal runs at
    1 elem/lane/cycle and was measured to cost ~1.8ms/core.
  * Node MLP with bias folded in via an appended ones-row (K=65 matmul).
Each core owns a contiguous 12544-node range; rows of its edges fall in that
range, so aggregation and the node MLP are fully local.
"""

import math
import numpy as np
import ml_dtypes

import concourse.bass as bass
import concourse.bacc as bacc
import concourse.tile as tile
from concourse import mybir
from concourse.tile import TileContext
from concourse.library_config import mlp as mlp_library

BF16 = mybir.dt.bfloat16
F32 = mybir.dt.float32
I16 = mybir.dt.int16
FP8 = mybir.dt.float8e4
NP_BF16 = ml_dtypes.bfloat16
NP_FP8 = ml_dtypes.float8_e4m3fn


class Cfg:
    def __init__(self, n_nodes, n_cores=8, spc=7, table_f32=False):
        self.n_swdge_queues = 4   # parallel SWDGE queues
        self.N = n_nodes
        self.n_cores = n_cores
        self.NPC = int(math.ceil(n_nodes / n_cores / 128)) * 128
        self.NP = self.NPC * n_cores
        self.W = self.NPC // 128          # windows per core
        self.C = 4                        # col chunks (int16 gather idx limit)
        assert self.NP % self.C == 0
        self.CHUNK = self.NP // self.C
        assert self.CHUNK <= 32767, "int16 gather index limit"
        # segments (windows) per gather call; must divide W
        self.SPC = spc
        assert self.W % self.SPC == 0
        self.CALLS_PER_CHUNK = self.W // self.SPC
        # idx loads cover IDX_CALLS gather calls each
        self.IDX_CALLS = self.CALLS_PER_CHUNK // 2 if self.CALLS_PER_CHUNK % 2 == 0 else self.CALLS_PER_CHUNK
        self.SEG = None  # set from data

    def stripe(self, total):
        for cand in (8192, 6272, 4096, 3136, 2048, 1792, 1568, 1024, 896, 784, 512, 448, 256, 128):
            if cand <= total and total % cand == 0:
                return cand
        raise AssertionError(total)


def build_kernel(cfg, phases=(0, 1, 2), p1_level=4, p2_level=3):
    """Build the single-core SPMD program. Returns nc.
    p1_level: 1=gathers only, 2=+onehot loads, 3=+matmuls/relu, 4=full."""
    SEG = cfg.SEG
    assert SEG is not None and SEG % 128 == 0
    EP = cfg.C * cfg.W * SEG               # padded edges per core
    NCALL = cfg.SPC * SEG                  # idxs per gather call
    JPC = NCALL // 128                     # 128-chunks per call
    JPS = SEG // 128                       # 128-chunks per segment
    # >64 descriptors/engine in one packet wedges the device; the per-engine
    # descriptor count is NCALL//16 + 1.
    SINGLE_PACKET = (NCALL // 16 + 1) <= 64

    NSWQ = getattr(cfg, "n_swdge_queues", 1)
    nc = bacc.Bacc("TRN2", target_bir_lowering=False, debug=False,
                   num_swdge_queues=NSWQ)

    # ---- DRAM I/O ----
    hTa_d = nc.dram_tensor("hTa", [65, cfg.NP], BF16, kind="ExternalInput")
    hTownb_d = nc.dram_tensor("hTownb", [65, cfg.NPC], BF16, kind="ExternalInput")
    hTown_d = nc.dram_tensor("hTown", [65, cfg.NPC], F32, kind="ExternalInput")
    waug_d = nc.dram_tensor("waug", [65, 128], BF16, kind="ExternalInput")
    nw1_d = nc.dram_tensor("nw1", [64, 64], F32, kind="ExternalInput")
    nw2a_d = nc.dram_tensor("nw2a", [65, 64], F32, kind="ExternalInput")
    ident_d = nc.dram_tensor("ident", [128, 128], BF16, kind="ExternalInput")
    colidx_d = nc.dram_tensor("colidx", [128, EP // 16], I16, kind="ExternalInput")
    oht_d = nc.dram_tensor("oht", [128, EP], FP8, kind="ExternalInput")
    oh2t_d = nc.dram_tensor("oh2t", [128, EP], FP8, kind="ExternalInput")
    # B table, flat [(NP+2)*64] bf16; node n's B at elements [n*64, n*64+64).
    # Gather views: elem 128 (256B) at idx*256B + group byte offset, so the
    # wanted node's 64 values always land in colg[..., 0:64] (no select).
    B_d = nc.dram_tensor("Btab", [(cfg.NP + 2) * 64], BF16)
    B_views = []
    for c in range(cfg.C):
        h, q = c // 2, c % 2
        base = h * cfg.CHUNK * 128 + q * 64
        B_views.append(
            B_d[base:base + cfg.CHUNK * 128].rearrange("(n e) -> n e", e=128))
    out_d = nc.dram_tensor("out", [cfg.NPC, 64], F32, kind="ExternalOutput")

    with TileContext(nc) as tc:
        nc.gpsimd.load_library(mlp_library)

        with tc.tile_pool(name="const", bufs=1) as cpool:
            waug_sb = cpool.tile([65, 128], BF16)
            nc.sync.dma_start(out=waug_sb[:], in_=waug_d[:])
            ident_sb = cpool.tile([128, 128], BF16)
            nc.sync.dma_start(out=ident_sb[:], in_=ident_d[:])
            nw1_sb = cpool.tile([64, 64], F32)
            nc.sync.dma_start(out=nw1_sb[:], in_=nw1_d[:])
            nw2a_sb = cpool.tile([65, 64], F32)
            nc.sync.dma_start(out=nw2a_sb[:], in_=nw2a_d[:])

            # aggT arena [65, NPC]: rows 0:64 = aggT, row 64 = ones (bias row)
            arena = cpool.tile([65, cfg.NPC], F32)
            nc.vector.memset(arena[64:65, :], 1.0)

            # A table for own nodes, SBUF-resident: [128, W, 64] bf16
            aown_sb = cpool.tile([128, cfg.W, 64], BF16)

            # ---- Phase 0: build AB table (all NP nodes) + Aown (SBUF) ----
            if 0 in phases:
                with tc.tile_pool(name="p0a", bufs=1) as p0a, \
                     tc.tile_pool(name="p0aps", bufs=4, space="PSUM") as p0aps:
                    htb = p0a.tile([65, cfg.NPC], BF16)
                    nc.sync.dma_start(out=htb[:], in_=hTownb_d[:])
                    for w in range(cfg.W):
                        ps = p0aps.tile([128, 64], F32)
                        nc.tensor.matmul(
                            out=ps[:], lhsT=htb[:, w * 128:(w + 1) * 128],
                            rhs=waug_sb[:, 0:64], start=True, stop=True)
                        nc.vector.tensor_copy(out=aown_sb[:, w, :], in_=ps[:])

                SN = cfg.stripe(cfg.NP // 4)
                JT = SN // 128
                B_rows = B_d[0:cfg.NP * 64].rearrange("(n f) -> n f", f=64)
                with tc.tile_pool(name="p0", bufs=2) as p0, \
                     tc.tile_pool(name="p0ps", bufs=4, space="PSUM") as p0ps:
                    for s in range(cfg.NP // SN):
                        hstripe = p0.tile([65, SN], BF16, tag="hstripe")
                        nc.sync.dma_start(
                            out=hstripe[:], in_=hTa_d[:, s * SN:(s + 1) * SN])
                        abst = p0.tile([128, JT, 64], BF16, tag="abst")
                        for j in range(JT):
                            ps = p0ps.tile([128, 64], F32)
                            nc.tensor.matmul(
                                out=ps[:], lhsT=hstripe[:, j * 128:(j + 1) * 128],
                                rhs=waug_sb[:, 64:128], start=True, stop=True)
                            nc.vector.tensor_copy(out=abst[:, j, :], in_=ps[:])
                        n0 = s * SN
                        nc.sync.dma_start(
                            out=B_rows[n0:n0 + SN, :].rearrange(
                                "(j p) f -> p j f", p=128),
                            in_=abst[:])

            # ---- Phase 1 + 2, interleaved by window group ----
            # Edge slots are ordered (window-group, chunk, window, intra) so
            # a group's 4 chunk-calls complete together and its node MLP
            # (phase 2) runs inline, overlapped with later groups' phase 1.
            if 1 in phases:
                with tc.tile_pool(name="idxp", bufs=2) as idxp, \
                     tc.tile_pool(name="gath", bufs=3) as gathp, \
                     tc.tile_pool(name="ohtp", bufs=2) as ohtp, \
                     tc.tile_pool(name="oh2tp", bufs=2) as oh2tp, \
                     tc.tile_pool(name="mp", bufs=3) as mp, \
                     tc.tile_pool(name="p2", bufs=2) as p2, \
                     tc.tile_pool(name="psA", bufs=3, space="PSUM") as psAp, \
                     tc.tile_pool(name="p1ps", bufs=3, space="PSUM") as p1ps, \
                     tc.tile_pool(name="p2ps", bufs=2, space="PSUM") as p2ps:
                    ILEN = cfg.C * NCALL // 16      # idx cols per group
                    for g in range(cfg.W // cfg.SPC):
                        cidx = idxp.tile([128, ILEN], I16, tag="cidx")
                        nc.sync.dma_start(
                            out=cidx[:],
                            in_=colidx_d[:, g * ILEN:(g + 1) * ILEN])
                        if 2 in phases:
                            hT2 = p2.tile([65, cfg.SPC * 128], F32, tag="hT2")
                            nc.sync.dma_start(
                                out=hT2[:],
                                in_=hTown_d[:, g * cfg.SPC * 128:(g + 1) * cfg.SPC * 128])
                        for c in range(cfg.C):
                            call = g * cfg.C + c
                            colg = gathp.tile([128, JPC, 128], BF16, tag="g")
                            nc.gpsimd.dma_gather(
                                colg[:], B_views[c],
                                cidx[:, c * (NCALL // 16):(c + 1) * (NCALL // 16)],
                                NCALL, NCALL, 128, single_packet=SINGLE_PACKET,
                                queue_num=call % NSWQ)
                            if p1_level < 2:
                                continue
                            e0 = call * NCALL
                            oht = ohtp.tile([128, JPC, 128], FP8, tag="oht")
                            nc.sync.dma_start(
                                out=oht[:],
                                in_=oht_d[:, e0:e0 + NCALL].rearrange(
                                    "p (a b) -> p a b", b=128))
                            oh2t = oh2tp.tile([128, JPC, 128], FP8, tag="oh2t")
                            nc.sync.dma_start(
                                out=oh2t[:],
                                in_=oh2t_d[:, e0:e0 + NCALL].rearrange(
                                    "p (a b) -> p a b", b=128))
                            for s in range(cfg.SPC if p1_level >= 3 else 0):
                                w = g * cfg.SPC + s
                                gseg = call * cfg.SPC + s
                                j0 = s * JPS
                                m2 = mp.tile([128, JPS, 64], BF16, tag="m2")
                                for jj in range(0, JPS, 3):
                                    gs = min(3, JPS - jj)
                                    pa = psAp.tile([128, gs, 64], F32)
                                    for j in range(gs):
                                        nc.tensor.matmul(
                                            out=pa[:, j, :],
                                            lhsT=oh2t[:, j0 + jj + j, :],
                                            rhs=aown_sb[:, w, :],
                                            start=True, stop=False)
                                        nc.tensor.matmul(
                                            out=pa[:, j, :],
                                            lhsT=ident_sb[:],
                                            rhs=colg[:, j0 + jj + j, 0:64],
                                            start=False, stop=True)
                                    # relu, alternating Act/DVE by segment
                                    if gseg % 2 == 0:
                                        nc.scalar.activation(
                                            out=m2[:, jj:jj + gs, :], in_=pa[:],
                                            func=mybir.ActivationFunctionType.Relu)
                                    else:
                                        nc.vector.tensor_scalar_max(
                                            m2[:, jj:jj + gs, :], pa[:], 0.0)
                                if p1_level < 4:
                                    continue
                                ps = p1ps.tile([64, 128], F32)
                                for j in range(JPS):
                                    nc.tensor.matmul(
                                        out=ps[:], lhsT=m2[:, j, :],
                                        rhs=oht[:, j0 + j, :],
                                        start=(j == 0), stop=(j == JPS - 1))
                                dst = arena[0:64, w * 128:(w + 1) * 128]
                                if c == 0:
                                    nc.vector.tensor_copy(out=dst, in_=ps[:])
                                else:
                                    nc.vector.tensor_tensor(
                                        out=dst, in0=dst, in1=ps[:],
                                        op=mybir.AluOpType.add)
                        # node MLP for this window group
                        if 2 in phases and p1_level >= 4:
                            ost = p2.tile([128, cfg.SPC, 64], F32, tag="ost")
                            for i in range(cfg.SPC):
                                w = g * cfg.SPC + i
                                if p2_level < 2:
                                    nc.vector.memset(ost[:, i, :], 0.0)
                                    continue
                                ps = p2ps.tile([128, 64], F32)
                                nc.tensor.matmul(
                                    out=ps[:], lhsT=hT2[0:64, i * 128:(i + 1) * 128],
                                    rhs=nw1_sb[:], start=True, stop=False)
                                nc.tensor.matmul(
                                    out=ps[:], lhsT=arena[:, w * 128:(w + 1) * 128],
                                    rhs=nw2a_sb[:], start=False, stop=True)
                                if p2_level < 3:
                                    nc.vector.tensor_copy(out=ost[:, i, :], in_=ps[:])
                                else:
                                    nc.scalar.activation(
                                        out=ost[:, i, :], in_=ps[:],
                                        func=mybir.ActivationFunctionType.Relu)
                            nc.sync.dma_start(
                                out=out_d[g * cfg.SPC * 128:(g + 1) * cfg.SPC * 128, :]
                                    .rearrange("(j p) f -> p j f", p=128),
                                in_=ost[:])

    nc.compile()
    return nc


# ---------------- host-side data prep ----------------

def _wrap16(a):
    x = np.ascontiguousarray(a.reshape(-1, 16).T)
    return np.tile(x, (8, 1))


def prep_inputs(cfg, h, edge_index, edge_w, edge_b, node_w, node_b):
    """Returns in_maps. Sets cfg.SEG."""
    N = cfg.N
    row = np.asarray(edge_index[0])
    col = np.asarray(edge_index[1])
    h = np.asarray(h, dtype=np.float32)

    # hT augmented with ones row, padded to NP cols
    hTa = np.zeros((65, cfg.NP), np.float32)
    hTa[:64, :N] = h.T
    hTa[64, :] = 1.0
    hTab = hTa.astype(NP_BF16)

    waug = np.zeros((65, 128), np.float32)
    waug[:64, 0:64] = edge_w[:64]
    waug[:64, 64:128] = edge_w[64:]
    waug[64, 0:64] = edge_b
    waugb = waug.astype(NP_BF16)

    nw1 = np.ascontiguousarray(node_w[:64], dtype=np.float32)
    nw2a = np.concatenate([node_w[64:], node_b[None, :]], axis=0).astype(np.float32)

    ident = np.eye(128, dtype=np.float32).astype(NP_BF16)

    # per-core edge prep; SEG = global max segment length (uniform program)
    per_core = []
    maxc = 1
    for k in range(cfg.n_cores):
        base = k * cfg.NPC
        m = (row >= base) & (row < base + cfg.NPC)
        r = (row[m] - base).astype(np.int64)
        c = col[m].astype(np.int64)
        w = r >> 7
        # pair-table group: (high half of col>>1, col&1)
        cc = (c >> 1) // cfg.CHUNK * 2 + (c & 1)
        # segment order: (window-group, chunk, window-within-group)
        seg_id = (w // cfg.SPC) * (cfg.C * cfg.SPC) + cc * cfg.SPC + (w % cfg.SPC)
        order = np.argsort(seg_id, kind="stable")
        r, c, seg_id = r[order], c[order], seg_id[order]
        counts = np.bincount(seg_id, minlength=cfg.C * cfg.W)
        if counts.size and r.size:
            maxc = max(maxc, int(counts.max()))
        per_core.append((r, c, seg_id, counts))
    SEG = int(math.ceil(maxc / 128.0)) * 128
    cfg.SEG = SEG
    EP = cfg.C * cfg.W * SEG

    nvals = np.arange(128, dtype=np.int16)
    in_maps = []
    for k in range(cfg.n_cores):
        r, c, seg_id, counts = per_core[k]
        starts = np.cumsum(counts) - counts
        intra = np.arange(r.size) - np.repeat(starts, counts)
        slots = seg_id * SEG + intra
        colidx = np.zeros(EP, np.int16)
        rl = np.full(EP, 255, np.int16)
        colidx[slots] = ((c >> 1) % cfg.CHUNK).astype(np.int16)
        rl[slots] = (r & 127).astype(np.int16)

        # fp8 one-hot indicator matrices (exact in fp8)
        ohf = (rl[:, None] == nvals[None, :]).astype(NP_FP8)   # [EP, 128]
        A3 = ohf.reshape(EP // 128, 128, 128)                  # [j, e', n]
        oht = np.ascontiguousarray(
            A3.transpose(1, 0, 2).reshape(128, EP))            # [p=e', j, n]
        oh2t = np.ascontiguousarray(
            A3.transpose(2, 0, 1).reshape(128, EP))            # [p=n, j, e']

        base = k * cfg.NPC
        hTown = np.ascontiguousarray(hTa[:, base:base + cfg.NPC])
        hTownb = np.ascontiguousarray(hTab[:, base:base + cfg.NPC])
        in_maps.append({
            "hTa": hTab,
            "hTownb": hTownb,
            "hTown": hTown,
            "waug": waugb,
            "nw1": nw1,
            "nw2a": nw2a,
            "ident": ident,
            "colidx": _wrap16(colidx),
            "oht": oht,
            "oh2t": oh2t,
        })
    return in_maps


def unshard_output(cfg, results):
    outs = [np.asarray(res["out"]) for res in results]
    full = np.concatenate(outs, axis=0)
    return np.ascontiguousarray(full[:cfg.N]).astype(np.float32)


# ---------------- entry point ----------------

def kernel(h, edge_index, edge_w, edge_b, node_w, node_b):
    from concourse.bass_utils import run_bass_kernel_spmd
    cfg = Cfg(n_nodes=100000, n_cores=8, spc=7)
    in_maps = prep_inputs(cfg, h, edge_index, edge_w, edge_b, node_w, node_b)
    nc = build_kernel(cfg)
    res = run_bass_kernel_spmd(nc, in_maps, core_ids=list(range(cfg.n_cores)))
    return unshard_output(cfg, res.results)
